# revision 10
# baseline (speedup 1.0000x reference)
"""Contrastive pair loss on 8 Trainium2 NeuronCores.

loss = mean_b( relu(mean_i((z1[b,i]-z2[b,i])^2) - margin) )  for
z1, z2 of shape (1024, 256, 16, 16) fp32.

Sharding: data-parallel over the batch axis — each of the 8 cores gets 128
rows (one row = 65536 contiguous fp32, 32 MiB per tensor per core).

On-chip layout: the shard is treated as 4096 chunks of 2048 floats. SBUF
partitions 0-123 each stream 33 consecutive chunks; partitions 124-127 get
one chunk each. This deliberately *unbalances* partitions: SDMA engine 15
(which serves partitions 92-95/124-127 and measures ~20% slower than its
peers) carries half the bytes of the other engines, so it stops pacing the
kernel. Chunks are 2048-aligned, so every accumulator slot (partition,
tile) covers elements of exactly one batch row; the host reassembles
per-row sums from the [128, 34] accumulator grid.

Per tile: DVE computes z1-z2 (in place over the z2 tile), ACT computes
Square with per-partition accumulation (accum_out); the full-size ACT
output is discarded through a stride-0 broadcast AP. The accumulator grid
is DMA'd out; hinge/mean run on host.
"""

import numpy as np

B = 1024
CODE = 256 * 16 * 16  # 65536 elements per row
N_CORES = 8
ROWS = B // N_CORES   # 128 rows per core
SHARD = ROWS * CODE   # 8388608 elements per core per tensor
G = 2048              # chunk size (divides CODE -> chunk never crosses a row)
NFAST = 124           # partitions with the bulk of the data
CFAST = 33            # chunks per fast partition
NSLOW = 4             # partitions 124..127 (engine 15's second half)
CSLOW = 1             # chunks per slow partition
assert NFAST * CFAST + NSLOW * CSLOW == SHARD // G
NSLOT = CFAST + 1     # accumulator slots per partition
SLOW_OFF = NFAST * CFAST * G  # element offset of the slow block
MARGIN = 0.01

_CACHE = {}


def _split_multi_waits(nc):
    """The walrus build in this image rejects instructions carrying more
    than one sync-wait command ("Too many sync wait commands",
    setupSyncWait). Tile routinely emits several waits on one instruction,
    so split them: for each instruction with N>1 waits, inject N-1
    single-wait NoOps on the same engine immediately before it. Same-engine
    program order makes this semantically identical."""
    from concourse import mybir

    k = 0
    for fn in nc.m.functions:
        for blk in fn.blocks:
            insts = blk.instructions
            out = []
            changed = False
            for ins in insts:
                si = ins.sync_info
                if si is not None and si.on_wait and len(si.on_wait) > 1:
                    waits = list(si.on_wait)
                    for w in waits[:-1]:
                        k += 1
                        nop = mybir.InstNoOp(
                            name=f"WSPLIT-{k}",
                            text_hint="split_wait",
                            bass_nofuse=True,
                        )
                        nop.engine = ins.engine
                        nop.sync_info = mybir.SyncInfo(on_wait=[w], on_update=[])
                        out.append(nop)
                    si.on_wait = waits[-1:]
                    ins.sync_info = si
                    changed = True
                out.append(ins)
            if changed:
                blk.instructions = out


def _patch_lean_epilogue():
    """Tile's kernel-tail epilogue is drain + EVSEM-butterfly barrier +
    sem clears + second butterfly. Replace the two full (drain+butterfly)
    barriers with sequencer-level sem-only barriers; DMA completion is
    already guaranteed by the drain's sem waits."""
    from concourse.tile import TileContext, ScopedClock

    if getattr(TileContext, "_ant_lean_epilogue", False):
        return

    def _drain_and_barrier(self, tick_clock, wait_clock):
        nc = self.nc
        drain_inst = nc.sync.drain()
        wait_clock.add_sem_waits(
            drain_inst.ins, ScopedClock({None: tick_clock.global_clock})
        )
        nc.all_engine_barrier(sem_only=True)
        assert self.sems is not None
        popped = nc._tile_sem_poison_stack.pop()
        assert popped is self._sem_poison
        nc.clear_and_free_semaphores(list(self.sems.allocated().values()))
        nc.all_engine_barrier(sem_only=True)

    TileContext._drain_and_barrier = _drain_and_barrier
    TileContext._ant_lean_epilogue = True


def _build():
    if "nc" in _CACHE:
        return _CACHE["nc"]

    import concourse.bass as bass
    from concourse import mybir
    from concourse.tile import TileContext

    _patch_lean_epilogue()

    nc = bass.Bass("TRN2", target_bir_lowering=False, num_devices=N_CORES)
    z1 = nc.dram_tensor("z1", [ROWS, CODE], mybir.dt.float32, kind="ExternalInput")
    z2 = nc.dram_tensor("z2", [ROWS, CODE], mybir.dt.float32, kind="ExternalInput")
    out = nc.dram_tensor(
        "out", [ROWS, NSLOT], mybir.dt.float32, kind="ExternalOutput"
    )

    def fast_ap(t, j):
        # partition p in 0..123 reads chunk p*CFAST+j: offset p*CFAST*G + j*G
        return bass.AP(t, j * G, [[CFAST * G, NFAST], [1, G]])

    def slow_ap(t):
        # partition q in 0..3 reads chunk NFAST*CFAST + q
        return bass.AP(t, SLOW_OFF, [[G, NSLOW], [1, G]])

    with TileContext(nc) as tc:
        with (
            tc.tile_pool(name="z1p", bufs=8) as p1,
            tc.tile_pool(name="z2p", bufs=8) as p2,
            tc.tile_pool(name="st", bufs=1) as ps,
        ):
            acc = ps.tile([ROWS, NSLOT], mybir.dt.float32)
            dummy = ps.tile([ROWS, 1], mybir.dt.float32)

            # slow block first: 4 partitions x 1 chunk (engine 15 relief)
            s1 = p1.tile([ROWS, G], mybir.dt.float32, tag="z1t")
            nc.sync.dma_start(out=s1[:NSLOW], in_=slow_ap(z1))
            s2 = p2.tile([ROWS, G], mybir.dt.float32, tag="z2t")
            nc.sync.dma_start(out=s2[:NSLOW], in_=slow_ap(z2))
            nc.vector.tensor_sub(
                out=s2[:NSLOW], in0=s1[:NSLOW], in1=s2[:NSLOW]
            )
            nc.scalar.activation(
                out=dummy[:NSLOW].broadcast_to((NSLOW, G)),
                in_=s2[:NSLOW],
                func=mybir.ActivationFunctionType.Square,
                accum_out=acc[:NSLOW, CFAST : CFAST + 1],
            )

            for j in range(CFAST):
                t1 = p1.tile([ROWS, G], mybir.dt.float32, tag="z1t")
                nc.sync.dma_start(out=t1[:NFAST], in_=fast_ap(z1, j))
                t2 = p2.tile([ROWS, G], mybir.dt.float32, tag="z2t")
                nc.sync.dma_start(out=t2[:NFAST], in_=fast_ap(z2, j))
                nc.vector.tensor_sub(
                    out=t2[:NFAST], in0=t1[:NFAST], in1=t2[:NFAST]
                )
                nc.scalar.activation(
                    out=dummy[:NFAST].broadcast_to((NFAST, G)),
                    in_=t2[:NFAST],
                    func=mybir.ActivationFunctionType.Square,
                    accum_out=acc[:NFAST, j : j + 1],
                )

            nc.sync.dma_start(out=out[:], in_=acc[:])

    _split_multi_waits(nc)

    _CACHE["nc"] = nc
    return nc


def _row_sums(acc_grid):
    """acc_grid: [128, NSLOT] per-core accumulator -> [128] row sums."""
    chunks = np.empty(SHARD // G, dtype=np.float64)
    p = np.arange(NFAST)
    chunks[(CFAST * p)[:, None] + np.arange(CFAST)[None, :]] = acc_grid[
        :NFAST, :CFAST
    ]
    chunks[NFAST * CFAST :] = acc_grid[:NSLOW, CFAST]
    return chunks.reshape(ROWS, CODE // G).sum(axis=1)


def _run(z1, z2, trace=False):
    from concourse.bass_utils import run_bass_kernel_spmd

    nc = _build()
    z1f = np.ascontiguousarray(np.asarray(z1, dtype=np.float32)).reshape(B, CODE)
    z2f = np.ascontiguousarray(np.asarray(z2, dtype=np.float32)).reshape(B, CODE)
    in_maps = [
        {
            "z1": z1f[c * ROWS : (c + 1) * ROWS],
            "z2": z2f[c * ROWS : (c + 1) * ROWS],
        }
        for c in range(N_CORES)
    ]
    res = run_bass_kernel_spmd(
        nc, in_maps, core_ids=list(range(N_CORES)), trace=trace
    )
    rowsum = np.concatenate(
        [_row_sums(res.results[c]["out"]) for c in range(N_CORES)]
    )
    hamm = rowsum / CODE
    hinged = np.where(hamm > MARGIN, hamm - MARGIN, 0.0)
    loss = np.float32(hinged.sum() / B)
    return np.asarray(loss, dtype=np.float32), res


def kernel(z1, z2):
    return _run(z1, z2, trace=False)[0]


# revision 11
# speedup vs baseline: 2.1390x; 2.1390x over previous
"""Contrastive pair loss on 8 Trainium2 NeuronCores.

loss = mean_b( relu(mean_i((z1[b,i]-z2[b,i])^2) - margin) )  for
z1, z2 of shape (1024, 256, 16, 16) fp32.

Sharding: data-parallel over the batch axis — each of the 8 cores gets 128
rows (one row = 65536 contiguous fp32, 32 MiB per tensor per core).

On-chip layout: the shard is treated as 2048 chunks of 4096 floats. SBUF
partitions 0-111 each stream 17 consecutive chunks; partitions 112-127
stream 9 each. This deliberately skews bytes away from SDMA engines
9/11/13/15 (which serve partitions 112-127 as half of their fixed
partition sets) — engine 15 measures ~20% slower than its peers and
otherwise paces the whole kernel. Partition counts are kept at 112/16/128,
whose largest divisor <=16 is 16, so every DMA sprays across all 16 SDMA
engines (a 124-partition DMA degrades to a 4-engine split).

Chunks are 4096-aligned, so every accumulator slot (partition, tile)
covers elements of exactly one batch row; the host reassembles per-row
sums from the [128, 17] accumulator grid.

Per tile: one fast DMA fills partitions 0-111 and (for j<9) one slow DMA
fills 112-127 of the same SBUF tile; DVE computes z1-z2 over all 128
partitions in place, ACT computes Square with per-partition accumulation
(accum_out), discarding its full-size output through a stride-0 broadcast
AP. Stale lanes in slots where the slow class has no data produce garbage
accumulator values that the host ignores. The accumulator grid is DMA'd
out; hinge/mean run on host.
"""

import numpy as np

B = 1024
CODE = 256 * 16 * 16  # 65536 elements per row
N_CORES = 8
ROWS = B // N_CORES   # 128 rows per core
SHARD = ROWS * CODE   # 8388608 elements per core per tensor
G = 4096              # chunk size (divides CODE -> chunk never crosses a row)
NFAST = 112           # partitions carrying the bulk
CFAST = 17            # chunks per fast partition
NSLOW = 16            # partitions 112..127
CSLOW = 9             # chunks per slow partition
assert NFAST * CFAST + NSLOW * CSLOW == SHARD // G
SLOW_OFF = NFAST * CFAST * G  # element offset of the slow block
MARGIN = 0.01

_CACHE = {}


def _split_multi_waits(nc):
    """The walrus build in this image rejects instructions carrying more
    than one sync-wait command ("Too many sync wait commands",
    setupSyncWait). Tile routinely emits several waits on one instruction,
    so split them: for each instruction with N>1 waits, inject N-1
    single-wait NoOps on the same engine immediately before it. Same-engine
    program order makes this semantically identical."""
    from concourse import mybir

    k = 0
    for fn in nc.m.functions:
        for blk in fn.blocks:
            insts = blk.instructions
            out = []
            changed = False
            for ins in insts:
                si = ins.sync_info
                if si is not None and si.on_wait and len(si.on_wait) > 1:
                    waits = list(si.on_wait)
                    for w in waits[:-1]:
                        k += 1
                        nop = mybir.InstNoOp(
                            name=f"WSPLIT-{k}",
                            text_hint="split_wait",
                            bass_nofuse=True,
                        )
                        nop.engine = ins.engine
                        nop.sync_info = mybir.SyncInfo(on_wait=[w], on_update=[])
                        out.append(nop)
                    si.on_wait = waits[-1:]
                    ins.sync_info = si
                    changed = True
                out.append(ins)
            if changed:
                blk.instructions = out


def _patch_lean_epilogue():
    """Tile's kernel-tail epilogue is drain + EVSEM-butterfly barrier +
    sem clears + second butterfly. Replace the two full (drain+butterfly)
    barriers with sequencer-level sem-only barriers; DMA completion is
    already guaranteed by the drain's sem waits."""
    from concourse.tile import TileContext, ScopedClock

    if getattr(TileContext, "_ant_lean_epilogue", False):
        return

    def _drain_and_barrier(self, tick_clock, wait_clock):
        nc = self.nc
        drain_inst = nc.sync.drain()
        wait_clock.add_sem_waits(
            drain_inst.ins, ScopedClock({None: tick_clock.global_clock})
        )
        nc.all_engine_barrier(sem_only=True)
        assert self.sems is not None
        popped = nc._tile_sem_poison_stack.pop()
        assert popped is self._sem_poison
        nc.clear_and_free_semaphores(list(self.sems.allocated().values()))
        nc.all_engine_barrier(sem_only=True)

    TileContext._drain_and_barrier = _drain_and_barrier
    TileContext._ant_lean_epilogue = True


def _build():
    if "nc" in _CACHE:
        return _CACHE["nc"]

    import concourse.bass as bass
    from concourse import mybir
    from concourse.tile import TileContext

    _patch_lean_epilogue()

    nc = bass.Bass("TRN2", target_bir_lowering=False, num_devices=N_CORES)
    z1 = nc.dram_tensor("z1", [ROWS, CODE], mybir.dt.float32, kind="ExternalInput")
    z2 = nc.dram_tensor("z2", [ROWS, CODE], mybir.dt.float32, kind="ExternalInput")
    out = nc.dram_tensor(
        "out", [ROWS, CFAST], mybir.dt.float32, kind="ExternalOutput"
    )

    def fast_ap(t, j):
        # partition p in 0..111 reads chunk p*CFAST+j
        return bass.AP(t, j * G, [[CFAST * G, NFAST], [1, G]])

    def slow_ap(t, j):
        # partition q in 0..15 reads chunk NFAST*CFAST + q*CSLOW + j
        return bass.AP(t, SLOW_OFF + j * G, [[CSLOW * G, NSLOW], [1, G]])

    with TileContext(nc) as tc:
        with (
            tc.tile_pool(name="z1p", bufs=5) as p1,
            tc.tile_pool(name="z2p", bufs=5) as p2,
            tc.tile_pool(name="st", bufs=1) as ps,
        ):
            acc = ps.tile([ROWS, CFAST], mybir.dt.float32)
            dummy = ps.tile([ROWS, 1], mybir.dt.float32)

            for j in range(CFAST):
                t1 = p1.tile([ROWS, G], mybir.dt.float32)
                nc.sync.dma_start(out=t1[:NFAST], in_=fast_ap(z1, j))
                t2 = p2.tile([ROWS, G], mybir.dt.float32)
                nc.sync.dma_start(out=t2[:NFAST], in_=fast_ap(z2, j))
                if j < CSLOW:
                    nc.sync.dma_start(out=t1[NFAST:ROWS], in_=slow_ap(z1, j))
                    nc.sync.dma_start(out=t2[NFAST:ROWS], in_=slow_ap(z2, j))
                nc.vector.tensor_sub(out=t2[:], in0=t1[:], in1=t2[:])
                nc.scalar.activation(
                    out=dummy[:].broadcast_to((ROWS, G)),
                    in_=t2[:],
                    func=mybir.ActivationFunctionType.Square,
                    accum_out=acc[:, j : j + 1],
                )

            nc.sync.dma_start(out=out[:], in_=acc[:])

    _split_multi_waits(nc)

    _CACHE["nc"] = nc
    return nc


def _row_sums(acc_grid):
    """acc_grid: [128, CFAST] per-core accumulator -> [128] row sums."""
    chunks = np.empty(SHARD // G, dtype=np.float64)
    p = np.arange(NFAST)
    chunks[(CFAST * p)[:, None] + np.arange(CFAST)[None, :]] = acc_grid[
        :NFAST, :CFAST
    ]
    q = np.arange(NSLOW)
    chunks[
        (NFAST * CFAST + CSLOW * q)[:, None] + np.arange(CSLOW)[None, :]
    ] = acc_grid[NFAST:ROWS, :CSLOW]
    return chunks.reshape(ROWS, CODE // G).sum(axis=1)


def _run(z1, z2, trace=False):
    from concourse.bass_utils import run_bass_kernel_spmd

    nc = _build()
    z1f = np.ascontiguousarray(np.asarray(z1, dtype=np.float32)).reshape(B, CODE)
    z2f = np.ascontiguousarray(np.asarray(z2, dtype=np.float32)).reshape(B, CODE)
    in_maps = [
        {
            "z1": z1f[c * ROWS : (c + 1) * ROWS],
            "z2": z2f[c * ROWS : (c + 1) * ROWS],
        }
        for c in range(N_CORES)
    ]
    res = run_bass_kernel_spmd(
        nc, in_maps, core_ids=list(range(N_CORES)), trace=trace
    )
    rowsum = np.concatenate(
        [_row_sums(res.results[c]["out"]) for c in range(N_CORES)]
    )
    hamm = rowsum / CODE
    hinged = np.where(hamm > MARGIN, hamm - MARGIN, 0.0)
    loss = np.float32(hinged.sum() / B)
    return np.asarray(loss, dtype=np.float32), res


def kernel(z1, z2):
    return _run(z1, z2, trace=False)[0]


# revision 12
# speedup vs baseline: 3.5424x; 1.6561x over previous
"""Contrastive pair loss on 8 Trainium2 NeuronCores.

loss = mean_b( relu(mean_i((z1[b,i]-z2[b,i])^2) - margin) )  for
z1, z2 of shape (1024, 256, 16, 16) fp32.

Sharding: data-parallel over the batch axis — each of the 8 cores gets 128
rows (one row = 65536 contiguous fp32, 32 MiB per tensor per core). On-chip,
each core streams the two shards through SBUF in [128, F] tiles: DVE
computes z1-z2 in place over the z2 tile, ACT computes Square with a
per-partition accumulation (accum_out) into one slot per tile, discarding
its full-size output through a stride-0 broadcast AP; a final DVE reduce
collapses the slots to per-row sums which are DMA'd out. The hinge/mean
epilogue over 1024 row values runs on host.

Structure choices driven by the trace:
- 4096-column body tiles keep every DMA a 2 MiB, 128-partition transfer
  (128 partitions are mandatory: partial-partition DMAs fan out to fewer
  SDMA engines and collapse throughput).
- The last tiles taper (2048/1024/1024) so the serial compute tail after
  the final load is ~2.5 us instead of ~8 us.
- Taper loads and the output DMA issue from the second HWDGE ring
  (nc.scalar) so they are not stuck behind the SP ring's descriptor
  backlog (the slowest SDMA engine backs up that ring's FIFO).
"""

import numpy as np

B = 1024
CODE = 256 * 16 * 16  # 65536
N_CORES = 8
ROWS = B // N_CORES  # 128 rows per core == SBUF partition count
TILES = [4096] * 15 + [2048, 1024, 1024]
NT = len(TILES)
MARGIN = 0.01

_CACHE = {}


def _split_multi_waits(nc):
    """The walrus build in this image rejects instructions carrying more
    than one sync-wait command ("Too many sync wait commands",
    setupSyncWait). Tile routinely emits several waits on one instruction,
    so split them: for each instruction with N>1 waits, inject N-1
    single-wait NoOps on the same engine immediately before it. Same-engine
    program order makes this semantically identical."""
    from concourse import mybir

    k = 0
    for fn in nc.m.functions:
        for blk in fn.blocks:
            insts = blk.instructions
            out = []
            changed = False
            for ins in insts:
                si = ins.sync_info
                if si is not None and si.on_wait and len(si.on_wait) > 1:
                    waits = list(si.on_wait)
                    for w in waits[:-1]:
                        k += 1
                        nop = mybir.InstNoOp(
                            name=f"WSPLIT-{k}",
                            text_hint="split_wait",
                            bass_nofuse=True,
                        )
                        nop.engine = ins.engine
                        nop.sync_info = mybir.SyncInfo(on_wait=[w], on_update=[])
                        out.append(nop)
                    si.on_wait = waits[-1:]
                    ins.sync_info = si
                    changed = True
                out.append(ins)
            if changed:
                blk.instructions = out


def _patch_lean_epilogue():
    """Tile's kernel-tail epilogue is drain + EVSEM-butterfly barrier +
    sem clears + second butterfly. Replace the two full (drain+butterfly)
    barriers with sequencer-level sem-only barriers; DMA completion is
    already guaranteed by the drain's sem waits."""
    from concourse.tile import TileContext, ScopedClock

    if getattr(TileContext, "_ant_lean_epilogue", False):
        return

    def _drain_and_barrier(self, tick_clock, wait_clock):
        nc = self.nc
        drain_inst = nc.sync.drain()
        wait_clock.add_sem_waits(
            drain_inst.ins, ScopedClock({None: tick_clock.global_clock})
        )
        nc.all_engine_barrier(sem_only=True)
        assert self.sems is not None
        popped = nc._tile_sem_poison_stack.pop()
        assert popped is self._sem_poison
        nc.clear_and_free_semaphores(list(self.sems.allocated().values()))
        nc.all_engine_barrier(sem_only=True)

    TileContext._drain_and_barrier = _drain_and_barrier
    TileContext._ant_lean_epilogue = True


def _build():
    if "nc" in _CACHE:
        return _CACHE["nc"]

    import concourse.bass as bass
    from concourse import mybir
    from concourse.tile import TileContext

    _patch_lean_epilogue()

    nc = bass.Bass("TRN2", target_bir_lowering=False, num_devices=N_CORES)
    z1 = nc.dram_tensor("z1", [ROWS, CODE], mybir.dt.float32, kind="ExternalInput")
    z2 = nc.dram_tensor("z2", [ROWS, CODE], mybir.dt.float32, kind="ExternalInput")
    out = nc.dram_tensor("out", [ROWS, 1], mybir.dt.float32, kind="ExternalOutput")

    with TileContext(nc) as tc:
        with (
            tc.tile_pool(name="z1p", bufs=5) as p1,
            tc.tile_pool(name="z2p", bufs=5) as p2,
            tc.tile_pool(name="st", bufs=1) as ps,
        ):
            acc = ps.tile([ROWS, NT], mybir.dt.float32)
            dummy = ps.tile([ROWS, 1], mybir.dt.float32)
            col = 0
            for j, f in enumerate(TILES):
                # taper loads go on the ACT HWDGE ring, body loads on SP's
                dge = nc.scalar if f < TILES[0] else nc.sync
                t1 = p1.tile([ROWS, TILES[0]], mybir.dt.float32)
                dge.dma_start(out=t1[:, :f], in_=z1[:, col : col + f])
                t2 = p2.tile([ROWS, TILES[0]], mybir.dt.float32)
                dge.dma_start(out=t2[:, :f], in_=z2[:, col : col + f])
                nc.vector.tensor_sub(out=t2[:, :f], in0=t1[:, :f], in1=t2[:, :f])
                nc.scalar.activation(
                    out=dummy[:].broadcast_to((ROWS, f)),
                    in_=t2[:, :f],
                    func=mybir.ActivationFunctionType.Square,
                    accum_out=acc[:, j : j + 1],
                )
                col += f
            rowsum = ps.tile([ROWS, 1], mybir.dt.float32)
            nc.vector.tensor_reduce(
                out=rowsum[:],
                in_=acc[:],
                axis=mybir.AxisListType.X,
                op=mybir.AluOpType.add,
            )
            nc.scalar.dma_start(out=out[:], in_=rowsum[:])

    _split_multi_waits(nc)

    _CACHE["nc"] = nc
    return nc


def _run(z1, z2, trace=False):
    from concourse.bass_utils import run_bass_kernel_spmd

    nc = _build()
    z1f = np.ascontiguousarray(np.asarray(z1, dtype=np.float32)).reshape(B, CODE)
    z2f = np.ascontiguousarray(np.asarray(z2, dtype=np.float32)).reshape(B, CODE)
    in_maps = [
        {
            "z1": z1f[c * ROWS : (c + 1) * ROWS],
            "z2": z2f[c * ROWS : (c + 1) * ROWS],
        }
        for c in range(N_CORES)
    ]
    res = run_bass_kernel_spmd(
        nc, in_maps, core_ids=list(range(N_CORES)), trace=trace
    )
    rowsum = np.concatenate(
        [res.results[c]["out"][:, 0] for c in range(N_CORES)]
    ).astype(np.float64)
    hamm = rowsum / CODE
    hinged = np.where(hamm > MARGIN, hamm - MARGIN, 0.0)
    loss = np.float32(hinged.sum() / B)
    return np.asarray(loss, dtype=np.float32), res


def kernel(z1, z2):
    return _run(z1, z2, trace=False)[0]
